# revision 6
# baseline (speedup 1.0000x reference)
"""Trainium2 Bass kernel for nn_Attention_21242908246407.

Computation (reference):
    uit = tanh(x @ W + b)          # [B,S,D]
    score = uit @ u[:,0]           # [B,S]
    weights = softmax(score, axis=1)
    context = einsum('bs,bsd->bd', weights, x)
    returns (context, weights)

Sharding: data-parallel over batch. B=64 across 8 cores -> 8 batches/core.
W/b/u replicated. Each core computes its shard fully independently (no
collectives); host concatenates.

Per-core pipeline (batch loop, s-tiles of 128 rows):
  - SWDGE DMA loads x cast fp32->bf16, natural layout [s=128p, d free].
  - PE transpose (matmul is_transpose) puts d on partitions -> xT in PSUM,
    copied to SBUF (DVE/ACT alternating).
  - z = xT.T @ W on PE (bf16, K=128 chunks, accumulated in PSUM fp32).
  - tanh on ACT: PSUM -> SBUF bf16.
  - score column = fused multiply+reduce on DVE against broadcast u.
  - softmax per batch with gpsimd.partition_all_reduce (scores are [128,32]).
  - context = sum_s w_s * x_s: 32 accumulating PSUM matmuls, w-columns as
    stationary [128,1], natural-layout x as moving operand.
"""

import numpy as np

B, S, D = 64, 4096, 256
N_CORES = 8
BPC = B // N_CORES          # batches per core
P = 128                     # partitions
ST = S // P                 # s-tiles per batch (32)
DC = D // P                 # d chunks (2)

_module_cache = {}


def _build_module(has_b: bool):
    import concourse.bass as bass
    import concourse.tile as tile
    import concourse.mybir as mybir
    from concourse import bacc, bass_isa
    from concourse.masks import make_identity

    dt = mybir.dt
    AF = mybir.ActivationFunctionType
    ALU = mybir.AluOpType
    AX = mybir.AxisListType

    nc = bacc.Bacc("TRN2", target_bir_lowering=False, debug=False,
                   num_devices=N_CORES)

    x_h = nc.dram_tensor("x", [BPC, S, D], dt.float32, kind="ExternalInput")
    W_h = nc.dram_tensor("W", [D, D], dt.float32, kind="ExternalInput")
    b_h = nc.dram_tensor("b", [D], dt.float32, kind="ExternalInput")
    u_h = nc.dram_tensor("u", [D, 1], dt.float32, kind="ExternalInput")
    ctx_h = nc.dram_tensor("ctx", [BPC, D], dt.float32, kind="ExternalOutput")
    wts_h = nc.dram_tensor("wts", [BPC, S], dt.float32, kind="ExternalOutput")

    x_ap = x_h.ap()
    W_ap = W_h.ap()
    b_ap = b_h.ap()
    u_ap = u_h.ap()
    ctx_ap = ctx_h.ap()
    wts_ap = wts_h.ap()

    with tile.TileContext(nc) as tc:
        import contextlib
        with contextlib.ExitStack() as ctx:
            singles = ctx.enter_context(tc.tile_pool(name="singles", bufs=1))
            x_pool = ctx.enter_context(tc.tile_pool(name="x", bufs=2))
            xT_pool = ctx.enter_context(tc.tile_pool(name="xT", bufs=3))
            uit_pool = ctx.enter_context(tc.tile_pool(name="uit", bufs=3))
            scr_pool = ctx.enter_context(tc.tile_pool(name="scr", bufs=4))
            sc_pool = ctx.enter_context(tc.tile_pool(name="sc", bufs=2))
            small = ctx.enter_context(tc.tile_pool(name="small", bufs=4))
            wout_pool = ctx.enter_context(tc.tile_pool(name="wout", bufs=2))
            psumT_pool = ctx.enter_context(
                tc.tile_pool(name="psT", bufs=2, space="PSUM"))
            z_pool = ctx.enter_context(
                tc.tile_pool(name="psZ", bufs=3, space="PSUM"))
            ctxp_pool = ctx.enter_context(
                tc.tile_pool(name="psC", bufs=1, space="PSUM"))
            wtp_pool = ctx.enter_context(
                tc.tile_pool(name="psW", bufs=2, space="PSUM"))

            # ---- constants / replicated params ----
            ident = singles.tile([P, P], dt.bfloat16)
            make_identity(nc, ident)
            ident32 = singles.tile([P, P], dt.float32)
            make_identity(nc, ident32)

            # W as bf16, d-chunks on partitions: [p, c, e]
            W_sb = singles.tile([P, DC, D], dt.bfloat16)
            nc.gpsimd.dma_start(
                out=W_sb,
                in_=W_ap.rearrange("(c p) e -> p c e", p=P))

            # u broadcast to all partitions, bf16 [128, 256]
            u_bc = singles.tile([P, D], dt.bfloat16)
            u_bcast_ap = bass.AP(u_ap.tensor, 0, [[0, P], [1, D]])
            nc.gpsimd.dma_start(out=u_bc, in_=u_bcast_ap)

            if has_b:
                ones_row = singles.tile([1, P], dt.bfloat16)
                nc.vector.memset(ones_row, 1.0)
                b_sb = singles.tile([1, D], dt.bfloat16)
                nc.gpsimd.dma_start(
                    out=b_sb, in_=bass.AP(b_ap.tensor, 0, [[0, 1], [1, D]]))

            for b in range(BPC):
                # ---- load batch (cast fp32 -> bf16), natural layout ----
                x_nat = x_pool.tile([P, ST, D], dt.bfloat16)
                xb = x_ap[b].rearrange("(i p) d -> p i d", p=P)
                for q in range(4):
                    nc.gpsimd.dma_start(
                        out=x_nat[:, q * 8:(q + 1) * 8, :],
                        in_=xb[:, q * 8:(q + 1) * 8, :])

                scores_b = sc_pool.tile([P, ST], dt.float32)

                # ---- phase 1: scores ----
                for g in range(8):          # groups of 4 s-tiles
                    xT_ps = psumT_pool.tile([P, DC, 4, P], dt.bfloat16)
                    for j in range(4):
                        i = g * 4 + j
                        for c in range(DC):
                            nc.tensor.matmul(
                                xT_ps[:, c, j, :],
                                lhsT=x_nat[:, i, c * P:(c + 1) * P],
                                rhs=ident,
                                is_transpose=True,
                                start=(j == 0 and c == 0),
                                stop=(j == 3 and c == DC - 1))
                    xT_sb = xT_pool.tile([P, DC, 4, P], dt.bfloat16)
                    if g % 2 == 0:
                        nc.vector.tensor_copy(xT_sb, xT_ps)
                    else:
                        nc.scalar.copy(xT_sb, xT_ps)

                    for h in range(2):      # z banks of 2 s-tiles
                        z_ps = z_pool.tile([P, 2, D], dt.float32)
                        # one accumulation group spans the whole bank:
                        # start on the first matmul, stop on the last.
                        mms = []
                        for j2 in range(2):
                            jj = h * 2 + j2
                            for c in range(DC):
                                mms.append((z_ps[:, j2, :],
                                            xT_sb[:, c, jj, :],
                                            W_sb[:, c, :]))
                        if has_b:
                            for j2 in range(2):
                                mms.append((z_ps[:, j2, :], ones_row, b_sb))
                        for k, (o, l, r) in enumerate(mms):
                            nc.tensor.matmul(
                                o, lhsT=l, rhs=r,
                                start=(k == 0), stop=(k == len(mms) - 1))
                        uit = uit_pool.tile([P, 2, D], dt.bfloat16)
                        nc.scalar.activation(uit, z_ps, AF.Tanh)
                        for j2 in range(2):
                            i = g * 4 + h * 2 + j2
                            prod = scr_pool.tile([P, D], dt.bfloat16)
                            nc.vector.scalar_tensor_tensor(
                                out=prod,
                                in0=uit[:, j2, :],
                                scalar=1.0,
                                in1=u_bc,
                                op0=ALU.mult,
                                op1=ALU.mult,
                                accum_out=scores_b[:, i:i + 1])

                # ---- softmax over the batch ----
                mx = small.tile([P, 1], dt.float32)
                nc.vector.reduce_max(mx, scores_b, axis=AX.X)
                mxr = small.tile([P, 1], dt.float32)
                nc.gpsimd.partition_all_reduce(
                    mxr, mx, P, bass_isa.ReduceOp.max)
                nmx = small.tile([P, 1], dt.float32)
                nc.vector.tensor_scalar_mul(nmx, mxr, -1.0)
                exp_b = sc_pool.tile([P, ST], dt.float32)
                nc.scalar.activation(exp_b, scores_b, AF.Exp,
                                     bias=nmx, scale=1.0)
                sm = small.tile([P, 1], dt.float32)
                nc.vector.reduce_sum(sm, exp_b, axis=AX.X)
                smr = small.tile([P, 1], dt.float32)
                nc.gpsimd.partition_all_reduce(
                    smr, sm, P, bass_isa.ReduceOp.add)
                rz = small.tile([P, 1], dt.float32)
                nc.vector.reciprocal(rz, smr)
                wts_b = sc_pool.tile([P, ST], dt.float32)
                nc.vector.tensor_scalar_mul(wts_b, exp_b, rz)
                wts_bf = sc_pool.tile([P, ST], dt.bfloat16)
                nc.vector.tensor_copy(wts_bf, wts_b)

                # ---- context: 32 accumulating matmuls ----
                ctx_ps = ctxp_pool.tile([1, D], dt.float32)
                for i in range(ST):
                    nc.tensor.matmul(
                        ctx_ps,
                        lhsT=wts_bf[:, i:i + 1],
                        rhs=x_nat[:, i, :],
                        start=(i == 0),
                        stop=(i == ST - 1))
                ctx_sb = small.tile([1, D], dt.float32)
                nc.vector.tensor_copy(ctx_sb, ctx_ps)
                nc.sync.dma_start(out=ctx_ap[b:b + 1, :], in_=ctx_sb)

                # ---- weights out: transpose [128,32] -> [32,128] ----
                wT_ps = wtp_pool.tile([ST, P], dt.float32)
                nc.tensor.matmul(wT_ps, lhsT=wts_b, rhs=ident32,
                                 is_transpose=True, start=True, stop=True)
                wT_sb = wout_pool.tile([ST, P], dt.float32)
                nc.vector.tensor_copy(wT_sb, wT_ps)
                nc.sync.dma_start(
                    out=wts_ap[b].rearrange("(i p) -> i p", p=P),
                    in_=wT_sb)

    nc.compile()
    return nc


def _get_module(has_b: bool):
    if has_b not in _module_cache:
        _module_cache[has_b] = _build_module(has_b)
    return _module_cache[has_b]


def kernel(x, W, b, u):
    from concourse.bass_utils import run_bass_kernel_spmd

    x = np.ascontiguousarray(np.asarray(x, dtype=np.float32))
    W = np.ascontiguousarray(np.asarray(W, dtype=np.float32))
    b = np.ascontiguousarray(np.asarray(b, dtype=np.float32))
    u = np.ascontiguousarray(np.asarray(u, dtype=np.float32))

    has_b = bool(np.any(b != 0.0))
    nc = _get_module(has_b)

    core_ids = list(range(N_CORES))
    in_maps = []
    for ci in core_ids:
        in_maps.append({
            "x": x[ci * BPC:(ci + 1) * BPC],
            "W": W,
            "b": b,
            "u": u,
        })
    res = run_bass_kernel_spmd(nc, in_maps, core_ids)
    context = np.concatenate([res.results[ci]["ctx"] for ci in core_ids], axis=0)
    weights = np.concatenate([res.results[ci]["wts"] for ci in core_ids], axis=0)
    return (context, weights)


# revision 16
# speedup vs baseline: 2.3935x; 2.3935x over previous
"""Trainium2 Bass kernel for nn_Attention_21242908246407.

Computation (reference):
    uit = tanh(x @ W + b)          # [B,S,D]
    score = uit @ u[:,0]           # [B,S]
    weights = softmax(score, axis=1)
    context = einsum('bs,bsd->bd', weights, x)
    returns (context, weights)

Sharding: data-parallel over batch. B=64 across 8 cores -> 8 batches/core.
W/b/u replicated. Each core computes its shard fully independently (no
collectives); host concatenates.

Per-core pipeline (per batch, s-tiles of 128 rows, groups of 4 tiles):
  - SWDGE DMA loads x cast fp32->bf16, natural layout [s=128p, d free].
  - DMA xbar transpose (dma_start_transpose, SBUF->SBUF bf16) puts d on
    partitions: out[p, 2i+c, s] = x[s, i, c*128+p] -- stacked xT chunks,
    one DMA per batch-quarter.
  - z = xT.T @ W on PE (bf16 in / fp32 PSUM out, K=128 chunks).
  - tanh on ACT over a whole 2-bank PSUM group (FD=1024) -> SBUF bf16.
  - score column = fused multiply+reduce (scalar_tensor_tensor) on DVE
    (optionally alternating with GPSIMD) against broadcast u.
  - softmax per batch via gpsimd.partition_all_reduce (scores sit [128,32]).
  - context = sum_s w_s * x_s: 32 accumulating PSUM matmuls with the
    normalized weight columns as the stationary operand.
  - batches are software-pipelined: softmax+context of batch b-1 are
    emitted after the score phase of batch b.
"""

import numpy as np

B, S, D = 64, 4096, 256
N_CORES = 8
BPC = B // N_CORES          # batches per core
P = 128                     # partitions
ST = S // P                 # s-tiles per batch (32)
DC = D // P                 # d chunks (2)
G = 4                       # s-tiles per group
NG = ST // G                # groups per batch (8)

_module_cache = {}


def _build_module(has_b: bool, repeat: int = 1, score_split: bool = False):
    import contextlib
    import concourse.bass as bass
    import concourse.tile as tile
    import concourse.mybir as mybir
    from concourse import bacc, bass_isa
    from concourse.masks import make_identity

    dt = mybir.dt
    AF = mybir.ActivationFunctionType
    ALU = mybir.AluOpType
    AX = mybir.AxisListType

    nc = bacc.Bacc("TRN2", target_bir_lowering=False, debug=False,
                   num_devices=N_CORES)

    x_h = nc.dram_tensor("x", [BPC, S, D], dt.float32, kind="ExternalInput")
    W_h = nc.dram_tensor("W", [D, D], dt.float32, kind="ExternalInput")
    b_h = nc.dram_tensor("b", [D], dt.float32, kind="ExternalInput")
    u_h = nc.dram_tensor("u", [D, 1], dt.float32, kind="ExternalInput")
    ctx_h = nc.dram_tensor("ctx", [BPC, D], dt.float32, kind="ExternalOutput")
    wts_h = nc.dram_tensor("wts", [BPC, S], dt.float32, kind="ExternalOutput")

    x_ap, W_ap, b_ap, u_ap = x_h.ap(), W_h.ap(), b_h.ap(), u_h.ap()
    ctx_ap, wts_ap = ctx_h.ap(), wts_h.ap()

    with tile.TileContext(nc) as tc:
        with contextlib.ExitStack() as ctx:
            singles = ctx.enter_context(tc.tile_pool(name="singles", bufs=1))
            x_pool = ctx.enter_context(tc.tile_pool(name="x", bufs=3))
            xT_pool = ctx.enter_context(tc.tile_pool(name="xT", bufs=2))
            uit_pool = ctx.enter_context(tc.tile_pool(name="uit", bufs=3))
            scr_pool = ctx.enter_context(tc.tile_pool(name="scr", bufs=4))
            sc_pool = ctx.enter_context(tc.tile_pool(name="sc", bufs=3))
            small = ctx.enter_context(tc.tile_pool(name="small", bufs=6))
            wout_pool = ctx.enter_context(tc.tile_pool(name="wout", bufs=2))
            z_pool = ctx.enter_context(
                tc.tile_pool(name="psZ", bufs=3, space="PSUM"))
            ctxp_pool = ctx.enter_context(
                tc.tile_pool(name="psC", bufs=1, space="PSUM"))
            wtp_pool = ctx.enter_context(
                tc.tile_pool(name="psW", bufs=1, space="PSUM"))

            # ---- constants / replicated params ----
            ident = singles.tile([P, P], dt.bfloat16)
            make_identity(nc, ident)
            ident32 = singles.tile([P, P], dt.float32)
            make_identity(nc, ident32)

            W_sb = singles.tile([P, DC, D], dt.bfloat16)
            nc.gpsimd.dma_start(
                out=W_sb, in_=W_ap.rearrange("(c p) e -> p c e", p=P))

            u_bc = singles.tile([P, D], dt.bfloat16)
            nc.gpsimd.dma_start(
                out=u_bc, in_=bass.AP(u_ap.tensor, 0, [[0, P], [1, D]]))

            if has_b:
                ones_row = singles.tile([1, P], dt.bfloat16)
                nc.vector.memset(ones_row, 1.0)
                b_sb = singles.tile([1, D], dt.bfloat16)
                nc.gpsimd.dma_start(
                    out=b_sb, in_=bass.AP(b_ap.tensor, 0, [[0, 1], [1, D]]))

            state = {}

            def emit_scores(r):
                b = r % BPC
                x_nat = x_pool.tile([P, ST, D], dt.bfloat16)
                xT_all = xT_pool.tile([P, ST * DC, P], dt.bfloat16)
                xb = x_ap[b].rearrange("(i p) d -> p i d", p=P)
                for q in range(4):
                    nc.gpsimd.dma_start(
                        out=x_nat[:, q * 8:(q + 1) * 8, :],
                        in_=xb[:, q * 8:(q + 1) * 8, :])
                    # xbar transpose: out[p, 2i+c, s] = x[s, i, c*128+p]
                    nc.sync.dma_start_transpose(
                        out=xT_all[:, q * 8 * DC:(q + 1) * 8 * DC, :],
                        in_=x_nat[:, q * 8:(q + 1) * 8, :].rearrange(
                            "p i d -> p (i d)"))

                scores_b = sc_pool.tile([P, ST], dt.float32)

                for g in range(NG):
                    # --- z matmuls: [128, G, 256] fp32 = 2 banks ---
                    z_ps = z_pool.tile([P, G, D], dt.float32)
                    for j in range(G):
                        i = g * G + j
                        for c in range(DC):
                            # zero region = 2KB = 2 s-tiles; start/stop per
                            # region (tiles {0,1} and {2,3})
                            nc.tensor.matmul(
                                z_ps[:, j, :],
                                lhsT=xT_all[:, DC * i + c, :],
                                rhs=W_sb[:, c, :],
                                start=(j % 2 == 0 and c == 0),
                                stop=(j % 2 == 1 and c == DC - 1
                                      and not has_b))
                        if has_b and j % 2 == 1:
                            for j2 in (j - 1, j):
                                nc.tensor.matmul(
                                    z_ps[:, j2, :], lhsT=ones_row, rhs=b_sb,
                                    start=False, stop=(j2 == j))

                    # --- tanh over the whole group (FD=1024) ---
                    uit = uit_pool.tile([P, G, D], dt.bfloat16)
                    nc.scalar.activation(uit, z_ps, AF.Tanh)

                    # --- score columns ---
                    for j in range(G):
                        i = g * G + j
                        prod = scr_pool.tile([P, D], dt.bfloat16)
                        eng = (nc.gpsimd if (score_split and j % 2 == 1)
                               else nc.vector)
                        eng.scalar_tensor_tensor(
                            out=prod,
                            in0=uit[:, j, :],
                            scalar=1.0,
                            in1=u_bc,
                            op0=ALU.mult,
                            op1=ALU.mult,
                            accum_out=scores_b[:, i:i + 1])

                state[r] = (x_nat, scores_b)

            def emit_softmax_ctx(r):
                b = r % BPC
                x_nat, scores_b = state.pop(r)

                mx = small.tile([P, 1], dt.float32)
                nc.vector.reduce_max(mx, scores_b, axis=AX.X)
                mxr = small.tile([P, 1], dt.float32)
                nc.gpsimd.partition_all_reduce(
                    mxr, mx, P, bass_isa.ReduceOp.max)
                nmx = small.tile([P, 1], dt.float32)
                nc.vector.tensor_scalar_mul(nmx, mxr, -1.0)
                exp_b = sc_pool.tile([P, ST], dt.float32)
                nc.scalar.activation(exp_b, scores_b, AF.Exp,
                                     bias=nmx, scale=1.0)
                sm = small.tile([P, 1], dt.float32)
                nc.vector.reduce_sum(sm, exp_b, axis=AX.X)
                smr = small.tile([P, 1], dt.float32)
                nc.gpsimd.partition_all_reduce(
                    smr, sm, P, bass_isa.ReduceOp.add)
                rz = small.tile([P, 1], dt.float32)
                nc.vector.reciprocal(rz, smr)
                wts_b = sc_pool.tile([P, ST], dt.float32)
                nc.vector.tensor_scalar_mul(wts_b, exp_b, rz)
                wts_bf = sc_pool.tile([P, ST], dt.bfloat16)
                nc.vector.tensor_copy(wts_bf, wts_b)

                # context: 32 accumulating matmuls
                ctx_ps = ctxp_pool.tile([1, D], dt.float32)
                for i in range(ST):
                    nc.tensor.matmul(
                        ctx_ps,
                        lhsT=wts_bf[:, i:i + 1],
                        rhs=x_nat[:, i, :],
                        start=(i == 0), stop=(i == ST - 1))
                ctx_sb = small.tile([1, D], dt.float32)
                nc.vector.tensor_copy(ctx_sb, ctx_ps)
                nc.scalar.dma_start(out=ctx_ap[b:b + 1, :], in_=ctx_sb)

                # weights out: transpose [128,32] -> [32,128]
                wT_ps = wtp_pool.tile([ST, P], dt.float32)
                nc.tensor.matmul(wT_ps, lhsT=wts_b, rhs=ident32,
                                 is_transpose=True, start=True, stop=True)
                wT_sb = wout_pool.tile([ST, P], dt.float32)
                nc.vector.tensor_copy(wT_sb, wT_ps)
                nc.scalar.dma_start(
                    out=wts_ap[b].rearrange("(i p) -> i p", p=P),
                    in_=wT_sb)

            def emit_all():
                for r in range(BPC):
                    emit_scores(r)
                    if r >= 1:
                        emit_softmax_ctx(r - 1)
                emit_softmax_ctx(BPC - 1)

            if repeat == 1:
                emit_all()
            else:
                # hw-timing aid: loop the whole body `repeat` times
                with tc.For_i(0, repeat, 1):
                    emit_all()

    nc.compile()
    return nc


def _get_module(has_b: bool, repeat: int = 1, score_split: bool = False):
    key = (has_b, repeat, score_split)
    if key not in _module_cache:
        _module_cache[key] = _build_module(has_b, repeat, score_split)
    return _module_cache[key]


def kernel(x, W, b, u):
    from concourse.bass_utils import run_bass_kernel_spmd

    x = np.ascontiguousarray(np.asarray(x, dtype=np.float32))
    W = np.ascontiguousarray(np.asarray(W, dtype=np.float32))
    b = np.ascontiguousarray(np.asarray(b, dtype=np.float32))
    u = np.ascontiguousarray(np.asarray(u, dtype=np.float32))

    has_b = bool(np.any(b != 0.0))
    nc = _get_module(has_b)

    core_ids = list(range(N_CORES))
    in_maps = []
    for ci in core_ids:
        in_maps.append({
            "x": x[ci * BPC:(ci + 1) * BPC],
            "W": W,
            "b": b,
            "u": u,
        })
    res = run_bass_kernel_spmd(nc, in_maps, core_ids)
    context = np.concatenate([res.results[ci]["ctx"] for ci in core_ids], axis=0)
    weights = np.concatenate([res.results[ci]["wts"] for ci in core_ids], axis=0)
    return (context, weights)
